# revision 42
# baseline (speedup 1.0000x reference)
"""TRN2 Bass kernel for GPT-2 style causal self-attention (B=4, S=2048, D=1024, H=16).

Sharding: 8 cores = 4 batches x 2 head-groups (8 heads each).
Each core computes qkv projections for its (batch, head-group), runs causal
attention for its 8 heads, computes a partial c_proj, then a pairwise
ReduceScatter (replica groups [[0,1],[2,3],[4,5],[6,7]]) sums the two
head-group partials and splits the token rows between the pair.

Everything is bf16 (x, all weights, activations; matmuls accumulate in
f32 PSUM): fp32 matmuls run fp32_mode=HIGH which disables FWL fast
weight loads for themselves AND the next matmul, so all-bf16 puts every
N=512 matmul at the ~216ns issue roofline. Softmax needs no
max-subtraction (scores bounded ~|2.7| at this scale); masked entries
are zeroed after exp by one DVE multiply with precomputed {0,1} mask
tiles; the softmax denominator rides along as a 65th ones-column of V
in the same AV matmul. Diagonal key tiles width-restrict the score
matmul, exp, and AV to the causally live queries (the mask multiply
stays full width and sanitizes the stale remainder). Attention is
software-pipelined (scores 2 tiles ahead of AV); a head pair's score
matmuls run concurrently on disjoint PE row groups via tile_position.
qk weights stay SBUF-resident, pre-rearranged host-side so their
one-time load is contiguous (wq/wk on separate DMA queues); x is
prefetched a full phase ahead. Filler rate is tuned per phase; dummy
matmuls pad attention(3) (otherwise ACT-paced with idle PE — the HAM
clock gate would halve the PE clock) and the tail collective drain.
Outputs leave as bf16 on the sync queue (an RS-completion wait on the
gpsimd queue would stall partition_broadcasts behind it; bf16->f32
cast DMAs are gpsimd-only, and the RS output is already bf16 so the
upcast happens on host). The tail waits on one 1MB ReduceScatter
(~20us FIXED collective cost regardless of size — splitting it buys
nothing).
"""
import sys
sys.path.insert(0, "/opt/trn_rl_repo")
import numpy as np

B, S, D, H, HD = 4, 2048, 1024, 16, 64
NCORES = 8
HPC = H // 2          # 8 heads per core
ACH = HPC * HD        # 512 local a-channels
P = 128
QCN = 4               # token chunks
QCS = S // QCN        # 512
FKT = D // P          # 8 feature k-tiles
VW = HPC * (HD + 1)   # 520: per-head 64 v-dims + ones column
SKEW = 3              # attention pipeline skew (score tiles ahead of AV)
SPL = 256             # split point of the last chunk (tail RS shrink)

_CACHE = {}


def _build():
    from concourse import bacc, tile, mybir
    f32 = mybir.dt.float32
    f32r = mybir.dt.float32r
    Exp = mybir.ActivationFunctionType.Exp

    nc = bacc.Bacc("TRN2", target_bir_lowering=False, debug=False,
                   num_devices=NCORES)
    xt_e = nc.dram_tensor("xt", [D, S], mybir.dt.bfloat16,
                           kind="ExternalInput")
    # wq/wk arrive pre-rearranged host-side as [P, 4*FKT*P] ("p (ct k c)")
    # so the per-ct loads are fully contiguous (4KB/partition descriptors
    # instead of 512B strided lines - 4x the DMA queue throughput)
    wq_e = nc.dram_tensor("wq", [P, 4 * FKT * P], mybir.dt.bfloat16,
                           kind="ExternalInput")
    wk_e = nc.dram_tensor("wk", [P, 4 * FKT * P], mybir.dt.bfloat16,
                           kind="ExternalInput")
    wv_e = nc.dram_tensor("wv", [D, ACH], mybir.dt.bfloat16,
                           kind="ExternalInput")
    wp_e = nc.dram_tensor("wp", [ACH, D], mybir.dt.bfloat16,
                           kind="ExternalInput")
    cm_e = nc.dram_tensor("cmask", [P, 4, 2 * QCS], mybir.dt.bfloat16,
                          kind="ExternalInput")
    out_e = nc.dram_tensor("outp", [S // 2, D], mybir.dt.bfloat16,
                           kind="ExternalOutput")
    rg = [[0, 1], [2, 3], [4, 5], [6, 7]]

    with tile.TileContext(nc) as tc:
        with tc.tile_pool(name="sb", bufs=1) as sb, \
             tc.tile_pool(name="pp", bufs=1, space="PSUM") as pp, \
             tc.tile_pool(name="dr", bufs=1, space="DRAM") as dr:

            bf16 = mybir.dt.bfloat16
            kT = [sb.tile([P, S], bf16, name=f"kTr{i}", tag="kT", bufs=4)
                  for i in range(4)]
            vx = [sb.tile([P, VW], bf16, name=f"vxr{i}", tag="vx", bufs=16)
                  for i in range(16)]
            wv_t = [sb.tile([P, ACH], bf16, name=f"wvr{i}", tag="wv", bufs=8)
                    for i in range(FKT)]
            wp_t = {(a, o): sb.tile([P, 512], bf16, name=f"wpr{a}_{o}",
                                    tag="wp", bufs=8)
                    for a in range(4) for o in range(2)}
            # causal mask tiles ({0,1} bf16), one per diagonal offset;
            # applied as a single DVE multiply (cheaper than two gpsimd
            # affine_selects and off the gpsimd critical path)
            cm_t = sb.tile([P, 4, 2 * QCS], bf16, name="cmt", tag="cmt",
                           bufs=1)
            # persistent qk weight tiles, loaded once during qkv(0)
            wqk_t = {(proj, ct): sb.tile([P, FKT, P], bf16,
                                         name=f"w{proj}c{ct}",
                                         tag=f"w{proj}c{ct}", bufs=1)
                     for proj in ("q", "k") for ct in range(4)}
            parts = [dr.tile([QCS, D], bf16, name=f"part{q}",
                             tag=f"pq{q}") for q in range(QCN)]
            rsos = [dr.tile([QCS // 2, D], bf16, name=f"rso{q}",
                            tag=f"rq{q}") for q in range(QCN)]

            qt_all = {}    # (qc, ct) -> tile
            at_all = {}    # qc -> [4 tiles]
            rs_insts = []

            xc_all = {}    # qc -> [8 tiles]

            def load_x_unit(qc):
                xc_all[qc] = [sb.tile([P, QCS], bf16, name=f"xc{qc}_{k}",
                                      tag="xc", bufs=16) for k in range(FKT)]

                def load_x():
                    for k in range(FKT):
                        nc.sync.dma_start(
                            out=xc_all[qc][k],
                            in_=xt_e.ap()[k * P:(k + 1) * P,
                                          qc * QCS:(qc + 1) * QCS])
                return load_x

            def qk_ct_units(qc, proj, w_e, ct):
                xc = xc_all[qc]
                w_c = wqk_t[proj, ct]
                if qc == 0:
                    def load_w(w_c=w_c, w_e=w_e, ct=ct, proj=proj):
                        # wq on the scalar queue, wk on the (idle this
                        # early) gpsimd queue: the two 2MB loads run in
                        # parallel instead of wk queuing behind all of wq
                        # (which stalled the k-projection)
                        eng = nc.scalar if proj == "q" else nc.gpsimd
                        eng.dma_start(
                            out=w_c,
                            in_=w_e.ap()[:, ct * FKT * P:
                                         (ct + 1) * FKT * P]
                                .rearrange("p (k c) -> p k c", c=P))
                    yield load_w
                mm_ps = pp.tile([P, QCS], f32,
                                name=f"{proj}ps{qc}_{ct}", tag="mm1",
                                bufs=2)
                for k in range(FKT):
                    def mm(k=k, mm_ps=mm_ps, w_c=w_c, xck=xc[k]):
                        nc.tensor.matmul(mm_ps[:, :], w_c[:, k, :],
                                         xck[:, :], start=(k == 0),
                                         stop=(k == FKT - 1))
                    yield mm
                if proj == "q":
                    qt = sb.tile([P, QCS], bf16, name=f"qt{qc}_{ct}",
                                 tag="qt", bufs=8)
                    qt_all[qc, ct] = qt

                    def cp(qt=qt, mm_ps=mm_ps):
                        nc.vector.tensor_copy(out=qt, in_=mm_ps)
                    yield cp
                else:
                    def cp(ct=ct, mm_ps=mm_ps):
                        nc.vector.tensor_copy(
                            out=kT[ct][:, qc * QCS:(qc + 1) * QCS],
                            in_=mm_ps)
                    yield cp

            def qkv_units(qc, part="all"):
                """Generator of emission closures for the qkv phase of qc.
                x for chunk qc was already loaded by the PREVIOUS phase (one
                full attention phase of lead time, so filler matmuls never
                stall on the x DMA); this phase prefetches x for qc+1.
                part="head"/"rest" split chunk 0's qkv so attention(0) can
                start after ct0 (attention(0, hp) only needs q/k ct=hp, so
                ct1-3 run as its fillers instead of serializing)."""
                if part == "v":
                    yield from qkv_v_units(qc)
                    return
                cts = {"head": (0,), "rest": (1, 2, 3)}.get(part, (0, 1, 2, 3))
                if qc == 0 and part != "rest":
                    yield load_x_unit(0)

                    def load_wv():
                        # sync queue only - wp must NOT ride along here:
                        # its scalar-queue DMAs would be enqueued ahead of
                        # the wq/wk loads and stall the k-projection ~15us
                        for k in range(FKT):
                            nc.sync.dma_start(
                                out=wv_t[k],
                                in_=wv_e.ap()[k * P:(k + 1) * P, :])
                        nc.sync.dma_start(out=cm_t, in_=cm_e.ap())
                    yield load_wv
                if qc + 1 < QCN and part != "rest":
                    yield load_x_unit(qc + 1)
                for proj, w_e in (("q", wq_e), ("k", wk_e)):
                    for ct in cts:
                        yield from qk_ct_units(qc, proj, w_e, ct)
                if qc == 0 and part != "head":
                    def load_wp():
                        for a in range(4):
                            for o in range(2):
                                nc.scalar.dma_start(
                                    out=wp_t[a, o],
                                    in_=wp_e.ap()[a * P:(a + 1) * P,
                                                  o * 512:(o + 1) * 512])
                    yield load_wp
                if part in ("all", "head"):
                    yield from qkv_v_units(qc)

            def qkv_v_units(qc):
                xc = xc_all[qc]
                for vt in range(4):
                    v_ps = pp.tile([P, ACH], f32, name=f"vps{qc}_{vt}",
                                   tag="mm1", bufs=2)
                    for k in range(FKT):
                        def mm(k=k, v_ps=v_ps, xck=xc[k], vt=vt):
                            nc.tensor.matmul(v_ps[:, :],
                                             xck[:, vt * P:(vt + 1) * P],
                                             wv_t[k][:, :], start=(k == 0),
                                             stop=(k == FKT - 1))
                        yield mm

                    def vcp(qc=qc, vt=vt, v_ps=v_ps):
                        vxt = vx[qc * 4 + vt]
                        v3 = vxt.rearrange("p (h w) -> p h w", w=HD + 1)
                        nc.gpsimd.memset(v3[:, :, HD:HD + 1], 1.0)
                        nc.vector.tensor_copy(
                            out=v3[:, :, 0:HD],
                            in_=v_ps.rearrange("p (h d) -> p h d", d=HD))
                    yield vcp

            def cproj_units(qc, t0=0, t1=4, rs_out=None):
                """Generator of closures for c_proj + RS of token tiles
                [t0, t1) of chunk qc. The last chunk is emitted as two
                halves so its first RS overlaps the second half's c_proj
                and the tail only waits on a small collective."""
                at_tiles = at_all[qc]
                for tt in range(t0, t1):
                    for oc in range(2):
                        po = pp.tile([P, 512], f32,
                                     name=f"po{qc}_{tt}_{oc}", tag="mm1",
                                     bufs=2)
                        for a in range(4):
                            def mm(a=a, po=po, tt=tt, oc=oc):
                                nc.tensor.matmul(
                                    po[:, :],
                                    at_tiles[a][:, tt * P:(tt + 1) * P],
                                    wp_t[a, oc][:, :],
                                    start=(a == 0), stop=(a == 3))
                            yield mm

                        def st_(qc=qc, tt=tt, oc=oc, po=po):
                            pst = sb.tile([P, 512], bf16,
                                          name=f"pst{qc}_{tt}_{oc}",
                                          tag="pst", bufs=2)
                            nc.vector.tensor_copy(out=pst, in_=po)
                            dst = parts[qc][tt * P:(tt + 1) * P,
                                            oc * 512:(oc + 1) * 512]
                            nc.gpsimd.dma_start(out=dst, in_=pst)
                        yield st_

                def rs_(qc=qc, t0=t0, t1=t1, rs_out=rs_out):
                    out_t = rsos[qc] if rs_out is None else rs_out
                    rs_insts.append(nc.gpsimd.collective_compute(
                        "ReduceScatter", mybir.AluOpType.add,
                        ins=[parts[qc][t0 * P:t1 * P, :].opt()],
                        outs=[out_t[:, :].opt()],
                        replica_groups=rg))
                yield rs_

            def emit_attention(qc, fillers, q0=0, q1=QCS, rate=2.2,
                               prereqs=None):
                """Emit attention for queries [q0, q1) of chunk qc,
                interleaving filler closures at ~rate units per pipeline
                step (just enough PE filler work to keep the HAM activity
                monitor warm without stretching the ACT-bound attention
                cadence). Leftovers run after. Score matmul and exp are
                width-restricted on diagonal key tiles (causal); the
                affine_select stays full width so it also zeroes the
                never-written below-diagonal region of pt."""
                cq0 = qc * QCS + q0      # global index of first query
                qlen = q1 - q0
                nkt = (qc * QCS + q1) // P
                fi = 0
                budget = 0.0
                if q0 == 0:
                    at_all[qc] = [sb.tile([P, QCS], bf16, name=f"at{qc}_{j}",
                                          tag="at", bufs=8) for j in range(4)]
                at_tiles = at_all[qc]
                for hp in range(4):
                    # per-hp prerequisite producers (e.g. chunk 0's q/k
                    # projections for this head pair) must be EMITTED before
                    # this hp's score matmuls so Tile sees the dependency;
                    # the scheduler still overlaps their execution with the
                    # previous hp's ACT-paced steps
                    for u in (prereqs or {}).get(hp, []):
                        u()
                    h_e, h_o = 2 * hp, 2 * hp + 1
                    acc = {}
                    for h, half in ((h_e, 0), (h_o, 64)):
                        acc[h] = pp.tile([65, QCS], f32, name=f"acc{qc}_{q0}_{h}",
                                         tag="acc", bufs=2)
                    pts = {}
                    for step in range(nkt + SKEW):
                        if step < nkt:
                            kt = step
                            off = max(0, kt * P - cq0)
                            # both heads' score tiles share one 2-bank PSUM
                            # tile; a single exp covers the pair. On
                            # diagonal key tiles (off>0) queries q<off are
                            # fully masked: score matmul and exp are width-
                            # restricted to [off, qlen); the stale pt region
                            # left behind is zeroed by the full-width mask
                            # multiply below.
                            st = pp.tile([P, 2 * QCS], f32,
                                         name=f"st{qc}_{q0}_{hp}_{kt}",
                                         tag="st", bufs=2)
                            for h, half in ((h_e, 0), (h_o, 64)):
                                nc.tensor.matmul(
                                    st[:, half * 8 + off:half * 8 + qlen],
                                    kT[hp][half:half + 64,
                                           kt * P:(kt + 1) * P],
                                    qt_all[qc, hp][half:half + 64,
                                                   q0 + off:q1],
                                    start=True, stop=True,
                                    tile_position=(half, 0))
                            pt = sb.tile([P, 2 * QCS], bf16,
                                         name=f"pt{qc}_{q0}_{hp}_{kt}",
                                         tag="pt", bufs=SKEW + 1)
                            st3 = st.rearrange("p (h q) -> p h q", q=QCS)
                            pt3 = pt.rearrange("p (h q) -> p h q", q=QCS)
                            nc.scalar.activation(out=pt3[:, :, off:qlen],
                                                 in_=st3[:, :, off:qlen],
                                                 func=Exp, scale=0.125)
                            if kt * P >= cq0:
                                nc.vector.tensor_tensor(
                                    out=pt[:, :], in0=pt[:, :],
                                    in1=cm_t[:, off // P, :],
                                    op=mybir.AluOpType.mult)
                            pts[kt] = pt
                        if step >= SKEW:
                            kt2 = step - SKEW
                            pt2 = pts.pop(kt2)
                            off2 = max(0, kt2 * P - cq0)
                            for h, half in ((h_e, 0), (h_o, 64)):
                                nc.tensor.matmul(
                                    acc[h][:, off2:qlen],
                                    vx[kt2][:, h * 65:(h + 1) * 65],
                                    pt2[:, half * 8 + off2:half * 8 + qlen],
                                    start=(kt2 == 0),
                                    stop=(kt2 == nkt - 1))
                        budget += rate
                        while fi < len(fillers) and budget >= 1.0:
                            fillers[fi]()
                            fi += 1
                            budget -= 1.0
                    for h, half in ((h_e, 0), (h_o, 64)):
                        rsum = sb.tile([1, QCS], f32, name=f"rsum{qc}_{q0}_{h}",
                                       tag="rs", bufs=2)
                        nc.vector.tensor_copy(out=rsum[:, 0:qlen],
                                              in_=acc[h][64:65, 0:qlen])
                        rs_t = sb.tile([1, QCS], f32, name=f"rst{qc}_{q0}_{h}",
                                       tag="rs2", bufs=2)
                        nc.vector.reciprocal_approx_fast(
                            out=rs_t[:, 0:qlen], in_=rsum[:, 0:qlen])
                        rb_t = sb.tile([64, QCS], f32, name=f"rb{qc}_{q0}_{h}",
                                       tag="rb", bufs=2)
                        nc.gpsimd.partition_broadcast(rb_t[:, 0:qlen],
                                                      rs_t[:, 0:qlen])
                        # the very last head pair's normalize gates the tail
                        # cproj: emit it in two column halves so cproj(3)'s
                        # first token tiles start during the second half
                        nsp = 2 if (qc == QCN - 1 and hp == 3) else 1
                        for sp in range(nsp):
                            lo = q0 + sp * qlen // nsp
                            hi = q0 + (sp + 1) * qlen // nsp
                            nc.vector.tensor_tensor(
                                out=at_tiles[hp][half:half + 64, lo:hi],
                                in0=acc[h][0:64, lo - q0:hi - q0],
                                in1=rb_t[:, lo - q0:hi - q0],
                                op=mybir.AluOpType.mult)
                while fi < len(fillers):
                    fillers[fi]()
                    fi += 1

            # PE warmup: dummy matmuls so the HAM clock gate is released
            # before the first real GEMM phase. 10 cold matmuls ≈ 4.3us
            # covers the ~3.4us HAM busy window without delaying qkv(0)
            # (bf16 halved the startup weight/x DMAs).
            # memset on DVE, not gpsimd: the gpsimd q7 path (after the
            # ~4us NEFF preamble) was gating the first warmup matmul
            wrm = sb.tile([P, QCS], bf16, name="wrm", tag="wrm", bufs=1)
            nc.vector.memset(wrm, 0.0)
            for w in range(10):
                wps = pp.tile([P, QCS], f32, name=f"wps{w}", tag="mm1",
                              bufs=2)
                nc.tensor.matmul(wps[:, :], wrm[:, 0:128], wrm[:, :],
                                 start=True, stop=True)

            # qkv(0) standalone, then attention(qc) interleaved with
            # qkv(qc+1) and cproj(qc-1)
            for u in qkv_units(0):
                u()
            # preload the exp table-set (~2.7us ACT_TABLE_LOAD) here, during
            # qkv(0)'s matmuls, instead of at attention(0)'s first real exp.
            # Emitted AFTER qkv(0) so the table load can't delay the wq/wp
            # DMA triggers that share the ACT instruction stream.
            wscr = sb.tile([1, 32], f32, name="wscr", tag="wscr", bufs=1)
            nc.scalar.activation(out=wscr, in_=wrm[0:1, 0:32], func=Exp,
                                 scale=0.125)
            for qc in range(QCN - 1):
                a = list(cproj_units(qc - 1)) if qc > 0 else []
                b = list(qkv_units(qc + 1))
                fillers = []
                while a or b:
                    if a:
                        fillers.append(a.pop(0))
                    if b:
                        fillers.append(b.pop(0))
                emit_attention(qc, fillers,
                               rate=(3.0, 2.6, 2.2)[qc])
            # last chunk: attention(3) is ACT-paced with only cproj(2) as
            # real filler work — it runs dry a third of the way in, PE duty
            # thins, and the HAM clock gate halves the PE clock (which then
            # makes the steps PE-bound at the throttled rate). Pad the
            # filler list with dependency-free dummy matmuls and spread
            # everything evenly (rate 1.0 over ~72 steps). The RS has ~20us
            # FIXED cost (size-independent) so splitting it buys nothing —
            # one RS for the whole chunk.
            def dummy_units(n, pfx):
                for w in range(n):
                    def mmk(w=w):
                        wps = pp.tile([P, QCS], f32, name=f"{pfx}{w}",
                                      tag="mm1", bufs=2)
                        nc.tensor.matmul(wps[:, :], wrm[:, 0:128], wrm[:, :],
                                         start=True, stop=True)
                    yield mmk
            emit_attention(QCN - 1, list(cproj_units(QCN - 2))
                           + list(dummy_units(24, "ka")), rate=1.0)
            for u in cproj_units(QCN - 1):
                u()
            # PE keepalive through the tail collective: dummy matmuls keep
            # the HAM clock at full speed while the RS and out DMAs drain
            for u in dummy_units(28, "tws"):
                u()
            # final copies of reduced shards (bf16, upcast on host — the RS
            # output is already bf16 so a f32 cast-DMA adds no precision) on
            # the SYNC queue: its x-prefetch DMAs are long done, so an RS-
            # completion wait here blocks nothing. (On the gpsimd queue the
            # chunk-2 wait stalled attention(3)'s partition_broadcasts for
            # ~19us.) Still pinned after the RS triggers so the scheduler
            # can't hoist a wait ahead of its collective's trigger.
            from concourse.tile import add_dep_helper
            for q in range(QCN):
                di = nc.sync.dma_start(
                    out=out_e.ap()[q * 256:(q + 1) * 256, :],
                    in_=rsos[q][:, :])
                anchor = rs_insts[QCN - 2] if q < QCN - 1 else rs_insts[-1]
                add_dep_helper(di.ins, anchor.ins, sync=False,
                               reason="order out DMAs after RS triggers")
    nc.compile()
    return nc


def _get_nc():
    if "nc" not in _CACHE:
        _CACHE["nc"] = _build()
    return _CACHE["nc"]


def _wlayout(w):
    """[D, ACH] -> [P, 4*FKT*P]: per 128-col tile ct, "(k p) c -> p (k c)"."""
    cts = []
    for ct in range(4):
        blk = w[:, ct * P:(ct + 1) * P].reshape(FKT, P, P)
        cts.append(blk.transpose(1, 0, 2).reshape(P, FKT * P))
    return np.ascontiguousarray(np.concatenate(cts, axis=1))

def _in_maps(x, c_attn_w, c_proj_w):
    import ml_dtypes
    ch = np.arange(P)[:, None]
    co = np.arange(QCS)[None, :]
    ms = []
    for j in range(4):
        m = (co >= j * P + ch).astype(np.float32)
        ms.append(np.concatenate([m, m], axis=1))
    cmask = np.stack(ms, axis=1).astype(ml_dtypes.bfloat16)
    maps = []
    for c in range(NCORES):
        b, g = c // 2, c % 2
        h0 = g * HPC
        cols = slice(h0 * HD, h0 * HD + ACH)
        maps.append({
            "xt": np.ascontiguousarray(x[b].T).astype(ml_dtypes.bfloat16),
            "wq": _wlayout(c_attn_w[:, :D][:, cols]).astype(ml_dtypes.bfloat16),
            "wk": _wlayout(c_attn_w[:, D:2 * D][:, cols]).astype(
                ml_dtypes.bfloat16),
            "wv": np.ascontiguousarray(c_attn_w[:, 2 * D:][:, cols]).astype(
                ml_dtypes.bfloat16),
            "wp": np.ascontiguousarray(
                c_proj_w[h0 * HD:h0 * HD + ACH, :]).astype(ml_dtypes.bfloat16),
            "cmask": cmask,
        })
    return maps


def _run(inputs, trace=False):
    from concourse.bass_utils import run_bass_kernel_spmd
    x = np.asarray(inputs["x"], np.float32)
    c_attn_w = np.asarray(inputs["c_attn_w"], np.float32)
    c_attn_b = np.asarray(inputs["c_attn_b"], np.float32)
    c_proj_w = np.asarray(inputs["c_proj_w"], np.float32)
    c_proj_b = np.asarray(inputs["c_proj_b"], np.float32)
    assert not np.any(c_attn_b), "nonzero c_attn_b not supported"

    nc = _get_nc()
    res = run_bass_kernel_spmd(nc, _in_maps(x, c_attn_w, c_proj_w),
                               core_ids=list(range(NCORES)), trace=trace)
    out = np.empty((B, S, D), np.float32)
    for c in range(NCORES):
        b, g = c // 2, c % 2
        o = np.asarray(res.results[c]["outp"]).astype(np.float32)
        for qc in range(QCN):
            tok = qc * QCS + g * 256
            out[b, tok:tok + 256, :] = o[qc * 256:(qc + 1) * 256]
    if np.any(c_proj_b):
        out += c_proj_b
    return out, res


def kernel(**inputs):
    out, _ = _run(inputs, trace=False)
    return out



# revision 44
# speedup vs baseline: 1.0186x; 1.0186x over previous
"""TRN2 Bass kernel for GPT-2 style causal self-attention (B=4, S=2048, D=1024, H=16).

Sharding: 8 cores = 4 batches x 2 head-groups (8 heads each).
Each core computes qkv projections for its (batch, head-group), runs causal
attention for its 8 heads, computes a partial c_proj, then a pairwise
ReduceScatter (replica groups [[0,1],[2,3],[4,5],[6,7]]) sums the two
head-group partials and splits the token rows between the pair.

Everything is bf16 (x, all weights, activations; matmuls accumulate in
f32 PSUM): fp32 matmuls run fp32_mode=HIGH which disables FWL fast
weight loads for themselves AND the next matmul, so all-bf16 puts every
N=512 matmul at the ~216ns issue roofline. Softmax needs no
max-subtraction (scores bounded ~|2.7| at this scale); masked entries
are zeroed after exp by one DVE multiply with precomputed {0,1} mask
tiles; the softmax denominator rides along as a 65th ones-column of V
in the same AV matmul. Diagonal key tiles width-restrict the score
matmul, exp, and AV to the causally live queries (the mask multiply
stays full width and sanitizes the stale remainder). Attention is
software-pipelined (scores 2 tiles ahead of AV); a head pair's score
matmuls run concurrently on disjoint PE row groups via tile_position.
qk weights stay SBUF-resident, pre-rearranged host-side so their
one-time load is contiguous (wq/wk on separate DMA queues); x is
prefetched a full phase ahead. Filler rate is tuned per phase; dummy
matmuls pad attention(3) (otherwise ACT-paced with idle PE — the HAM
clock gate would halve the PE clock) and the tail collective drain.
Outputs leave as bf16 on the sync queue (an RS-completion wait on the
gpsimd queue would stall partition_broadcasts behind it; bf16->f32
cast DMAs are gpsimd-only, and the RS output is already bf16 so the
upcast happens on host). The tail waits on one 1MB ReduceScatter
(~20us FIXED collective cost regardless of size — splitting it buys
nothing).
"""
import sys
sys.path.insert(0, "/opt/trn_rl_repo")
import numpy as np

B, S, D, H, HD = 4, 2048, 1024, 16, 64
NCORES = 8
HPC = H // 2          # 8 heads per core
ACH = HPC * HD        # 512 local a-channels
P = 128
QCN = 4               # token chunks
QCS = S // QCN        # 512
FKT = D // P          # 8 feature k-tiles
VW = HPC * (HD + 1)   # 520: per-head 64 v-dims + ones column
SKEW = 3              # attention pipeline skew (score tiles ahead of AV)
SPL = 256             # split point of the last chunk (tail RS shrink)

_CACHE = {}


def _build():
    from concourse import bacc, tile, mybir
    f32 = mybir.dt.float32
    f32r = mybir.dt.float32r
    Exp = mybir.ActivationFunctionType.Exp

    nc = bacc.Bacc("TRN2", target_bir_lowering=False, debug=False,
                   num_devices=NCORES)
    xt_e = nc.dram_tensor("xt", [D, S], mybir.dt.bfloat16,
                           kind="ExternalInput")
    # wq/wk arrive pre-rearranged host-side as [P, 4*FKT*P] ("p (ct k c)")
    # so the per-ct loads are fully contiguous (4KB/partition descriptors
    # instead of 512B strided lines - 4x the DMA queue throughput)
    wq_e = nc.dram_tensor("wq", [P, 4 * FKT * P], mybir.dt.bfloat16,
                           kind="ExternalInput")
    wk_e = nc.dram_tensor("wk", [P, 4 * FKT * P], mybir.dt.bfloat16,
                           kind="ExternalInput")
    wv_e = nc.dram_tensor("wv", [D, ACH], mybir.dt.bfloat16,
                           kind="ExternalInput")
    wp_e = nc.dram_tensor("wp", [ACH, D], mybir.dt.bfloat16,
                           kind="ExternalInput")
    cm_e = nc.dram_tensor("cmask", [P, 4, 2 * QCS], mybir.dt.bfloat16,
                          kind="ExternalInput")
    out_e = nc.dram_tensor("outp", [S // 2, D], mybir.dt.bfloat16,
                           kind="ExternalOutput")
    rg = [[0, 1], [2, 3], [4, 5], [6, 7]]

    with tile.TileContext(nc) as tc:
        with tc.tile_pool(name="sb", bufs=1) as sb, \
             tc.tile_pool(name="pp", bufs=1, space="PSUM") as pp, \
             tc.tile_pool(name="dr", bufs=1, space="DRAM") as dr:

            bf16 = mybir.dt.bfloat16
            kT = [sb.tile([P, S], bf16, name=f"kTr{i}", tag="kT", bufs=4)
                  for i in range(4)]
            vx = [sb.tile([P, VW], bf16, name=f"vxr{i}", tag="vx", bufs=16)
                  for i in range(16)]
            wv_t = [sb.tile([P, ACH], bf16, name=f"wvr{i}", tag="wv", bufs=8)
                    for i in range(FKT)]
            wp_t = {(a, o): sb.tile([P, 512], bf16, name=f"wpr{a}_{o}",
                                    tag="wp", bufs=8)
                    for a in range(4) for o in range(2)}
            # causal mask tiles ({0,1} bf16), one per diagonal offset;
            # applied as a single DVE multiply (cheaper than two gpsimd
            # affine_selects and off the gpsimd critical path)
            cm_t = sb.tile([P, 4, 2 * QCS], bf16, name="cmt", tag="cmt",
                           bufs=1)
            # persistent qk weight tiles, loaded once during qkv(0)
            wqk_t = {(proj, ct): sb.tile([P, FKT, P], bf16,
                                         name=f"w{proj}c{ct}",
                                         tag=f"w{proj}c{ct}", bufs=1)
                     for proj in ("q", "k") for ct in range(4)}
            parts = [dr.tile([QCS, D], bf16, name=f"part{q}",
                             tag=f"pq{q}") for q in range(QCN)]
            rsos = [dr.tile([QCS // 2, D], bf16, name=f"rso{q}",
                            tag=f"rq{q}") for q in range(QCN)]

            qt_all = {}    # (qc, ct) -> tile
            at_all = {}    # qc -> [4 tiles]
            rs_insts = []

            xc_all = {}    # qc -> [8 tiles]

            def load_x_unit(qc):
                xc_all[qc] = [sb.tile([P, QCS], bf16, name=f"xc{qc}_{k}",
                                      tag="xc", bufs=16) for k in range(FKT)]

                def load_x():
                    for k in range(FKT):
                        nc.sync.dma_start(
                            out=xc_all[qc][k],
                            in_=xt_e.ap()[k * P:(k + 1) * P,
                                          qc * QCS:(qc + 1) * QCS])
                return load_x

            def qk_ct_units(qc, proj, w_e, ct):
                xc = xc_all[qc]
                w_c = wqk_t[proj, ct]
                if qc == 0:
                    def load_w(w_c=w_c, w_e=w_e, ct=ct, proj=proj):
                        # wq on the scalar queue, wk on the (idle this
                        # early) gpsimd queue: the two 2MB loads run in
                        # parallel instead of wk queuing behind all of wq
                        # (which stalled the k-projection)
                        eng = nc.scalar if proj == "q" else nc.gpsimd
                        eng.dma_start(
                            out=w_c,
                            in_=w_e.ap()[:, ct * FKT * P:
                                         (ct + 1) * FKT * P]
                                .rearrange("p (k c) -> p k c", c=P))
                    yield load_w
                mm_ps = pp.tile([P, QCS], f32,
                                name=f"{proj}ps{qc}_{ct}", tag="mm1",
                                bufs=2)
                for k in range(FKT):
                    def mm(k=k, mm_ps=mm_ps, w_c=w_c, xck=xc[k]):
                        nc.tensor.matmul(mm_ps[:, :], w_c[:, k, :],
                                         xck[:, :], start=(k == 0),
                                         stop=(k == FKT - 1))
                    yield mm
                if proj == "q":
                    qt = sb.tile([P, QCS], bf16, name=f"qt{qc}_{ct}",
                                 tag="qt", bufs=8)
                    qt_all[qc, ct] = qt

                    def cp(qt=qt, mm_ps=mm_ps):
                        nc.vector.tensor_copy(out=qt, in_=mm_ps)
                    yield cp
                else:
                    def cp(ct=ct, mm_ps=mm_ps):
                        nc.vector.tensor_copy(
                            out=kT[ct][:, qc * QCS:(qc + 1) * QCS],
                            in_=mm_ps)
                    yield cp

            def qkv_units(qc, part="all"):
                """Generator of emission closures for the qkv phase of qc.
                x for chunk qc was already loaded by the PREVIOUS phase (one
                full attention phase of lead time, so filler matmuls never
                stall on the x DMA); this phase prefetches x for qc+1.
                part="head"/"rest" split chunk 0's qkv so attention(0) can
                start after ct0 (attention(0, hp) only needs q/k ct=hp, so
                ct1-3 run as its fillers instead of serializing)."""
                if part == "v":
                    yield from qkv_v_units(qc)
                    return
                cts = {"head": (0,), "rest": (1, 2, 3)}.get(part, (0, 1, 2, 3))
                if qc == 0 and part != "rest":
                    yield load_x_unit(0)

                    def load_wv():
                        # sync queue only - wp must NOT ride along here:
                        # its scalar-queue DMAs would be enqueued ahead of
                        # the wq/wk loads and stall the k-projection ~15us
                        for k in range(FKT):
                            nc.sync.dma_start(
                                out=wv_t[k],
                                in_=wv_e.ap()[k * P:(k + 1) * P, :])
                        nc.sync.dma_start(out=cm_t, in_=cm_e.ap())
                    yield load_wv
                if qc + 1 < QCN and part != "rest":
                    yield load_x_unit(qc + 1)
                for proj, w_e in (("q", wq_e), ("k", wk_e)):
                    for ct in cts:
                        yield from qk_ct_units(qc, proj, w_e, ct)
                if qc == 0 and part != "head":
                    def load_wp():
                        for a in range(4):
                            for o in range(2):
                                nc.scalar.dma_start(
                                    out=wp_t[a, o],
                                    in_=wp_e.ap()[a * P:(a + 1) * P,
                                                  o * 512:(o + 1) * 512])
                    yield load_wp
                if part in ("all", "head"):
                    yield from qkv_v_units(qc)

            def qkv_v_units(qc):
                xc = xc_all[qc]
                for vt in range(4):
                    v_ps = pp.tile([P, ACH], f32, name=f"vps{qc}_{vt}",
                                   tag="mm1", bufs=2)
                    for k in range(FKT):
                        def mm(k=k, v_ps=v_ps, xck=xc[k], vt=vt):
                            nc.tensor.matmul(v_ps[:, :],
                                             xck[:, vt * P:(vt + 1) * P],
                                             wv_t[k][:, :], start=(k == 0),
                                             stop=(k == FKT - 1))
                        yield mm

                    def vcp(qc=qc, vt=vt, v_ps=v_ps):
                        vxt = vx[qc * 4 + vt]
                        v3 = vxt.rearrange("p (h w) -> p h w", w=HD + 1)
                        nc.gpsimd.memset(v3[:, :, HD:HD + 1], 1.0)
                        nc.vector.tensor_copy(
                            out=v3[:, :, 0:HD],
                            in_=v_ps.rearrange("p (h d) -> p h d", d=HD))
                    yield vcp

            def cproj_units(qc, t0=0, t1=4, rs_out=None):
                """Generator of closures for c_proj + RS of token tiles
                [t0, t1) of chunk qc. The last chunk is emitted as two
                halves so its first RS overlaps the second half's c_proj
                and the tail only waits on a small collective."""
                at_tiles = at_all[qc]
                for tt in range(t0, t1):
                    for oc in range(2):
                        po = pp.tile([P, 512], f32,
                                     name=f"po{qc}_{tt}_{oc}", tag="mm1",
                                     bufs=2)
                        for a in range(4):
                            def mm(a=a, po=po, tt=tt, oc=oc):
                                nc.tensor.matmul(
                                    po[:, :],
                                    at_tiles[a][:, tt * P:(tt + 1) * P],
                                    wp_t[a, oc][:, :],
                                    start=(a == 0), stop=(a == 3))
                            yield mm

                        def st_(qc=qc, tt=tt, oc=oc, po=po):
                            pst = sb.tile([P, 512], bf16,
                                          name=f"pst{qc}_{tt}_{oc}",
                                          tag="pst", bufs=2)
                            nc.vector.tensor_copy(out=pst, in_=po)
                            dst = parts[qc][tt * P:(tt + 1) * P,
                                            oc * 512:(oc + 1) * 512]
                            nc.gpsimd.dma_start(out=dst, in_=pst)
                        yield st_

                def rs_(qc=qc, t0=t0, t1=t1, rs_out=rs_out):
                    out_t = rsos[qc] if rs_out is None else rs_out
                    rs_insts.append(nc.gpsimd.collective_compute(
                        "ReduceScatter", mybir.AluOpType.add,
                        ins=[parts[qc][t0 * P:t1 * P, :].opt()],
                        outs=[out_t[:, :].opt()],
                        replica_groups=rg))
                yield rs_

            def emit_attention(qc, fillers, q0=0, q1=QCS, rate=2.2,
                               prereqs=None):
                """Emit attention for queries [q0, q1) of chunk qc,
                interleaving filler closures at ~rate units per pipeline
                step (just enough PE filler work to keep the HAM activity
                monitor warm without stretching the ACT-bound attention
                cadence). Leftovers run after. Score matmul and exp are
                width-restricted on diagonal key tiles (causal); the
                affine_select stays full width so it also zeroes the
                never-written below-diagonal region of pt."""
                cq0 = qc * QCS + q0      # global index of first query
                qlen = q1 - q0
                nkt = (qc * QCS + q1) // P
                fi = 0
                budget = 0.0
                if q0 == 0:
                    at_all[qc] = [sb.tile([P, QCS], bf16, name=f"at{qc}_{j}",
                                          tag="at", bufs=8) for j in range(4)]
                at_tiles = at_all[qc]
                for hp in range(4):
                    # per-hp prerequisite producers (e.g. chunk 0's q/k
                    # projections for this head pair) must be EMITTED before
                    # this hp's score matmuls so Tile sees the dependency;
                    # the scheduler still overlaps their execution with the
                    # previous hp's ACT-paced steps
                    for u in (prereqs or {}).get(hp, []):
                        u()
                    h_e, h_o = 2 * hp, 2 * hp + 1
                    acc = {}
                    for h, half in ((h_e, 0), (h_o, 64)):
                        acc[h] = pp.tile([65, QCS], f32, name=f"acc{qc}_{q0}_{h}",
                                         tag="acc", bufs=2)
                    pts = {}
                    for step in range(nkt + SKEW):
                        if step < nkt:
                            kt = step
                            off = max(0, kt * P - cq0)
                            # both heads' score tiles share one 2-bank PSUM
                            # tile; a single exp covers the pair. On
                            # diagonal key tiles (off>0) queries q<off are
                            # fully masked: score matmul and exp are width-
                            # restricted to [off, qlen); the stale pt region
                            # left behind is zeroed by the full-width mask
                            # multiply below.
                            st = pp.tile([P, 2 * QCS], f32,
                                         name=f"st{qc}_{q0}_{hp}_{kt}",
                                         tag="st", bufs=2)
                            for h, half in ((h_e, 0), (h_o, 64)):
                                nc.tensor.matmul(
                                    st[:, half * 8 + off:half * 8 + qlen],
                                    kT[hp][half:half + 64,
                                           kt * P:(kt + 1) * P],
                                    qt_all[qc, hp][half:half + 64,
                                                   q0 + off:q1],
                                    start=True, stop=True,
                                    tile_position=(half, 0))
                            pt = sb.tile([P, 2 * QCS], bf16,
                                         name=f"pt{qc}_{q0}_{hp}_{kt}",
                                         tag="pt", bufs=SKEW + 1)
                            st3 = st.rearrange("p (h q) -> p h q", q=QCS)
                            pt3 = pt.rearrange("p (h q) -> p h q", q=QCS)
                            nc.scalar.activation(out=pt3[:, :, off:qlen],
                                                 in_=st3[:, :, off:qlen],
                                                 func=Exp, scale=0.125)
                            if kt * P >= cq0:
                                nc.vector.tensor_tensor(
                                    out=pt[:, :], in0=pt[:, :],
                                    in1=cm_t[:, off // P, :],
                                    op=mybir.AluOpType.mult)
                            pts[kt] = pt
                        if step >= SKEW:
                            kt2 = step - SKEW
                            pt2 = pts.pop(kt2)
                            off2 = max(0, kt2 * P - cq0)
                            for h, half in ((h_e, 0), (h_o, 64)):
                                nc.tensor.matmul(
                                    acc[h][:, off2:qlen],
                                    vx[kt2][:, h * 65:(h + 1) * 65],
                                    pt2[:, half * 8 + off2:half * 8 + qlen],
                                    start=(kt2 == 0),
                                    stop=(kt2 == nkt - 1))
                        budget += rate
                        while fi < len(fillers) and budget >= 1.0:
                            fillers[fi]()
                            fi += 1
                            budget -= 1.0
                    for h, half in ((h_e, 0), (h_o, 64)):
                        rsum = sb.tile([1, QCS], f32, name=f"rsum{qc}_{q0}_{h}",
                                       tag="rs", bufs=2)
                        nc.vector.tensor_copy(out=rsum[:, 0:qlen],
                                              in_=acc[h][64:65, 0:qlen])
                        rs_t = sb.tile([1, QCS], f32, name=f"rst{qc}_{q0}_{h}",
                                       tag="rs2", bufs=2)
                        nc.vector.reciprocal_approx_fast(
                            out=rs_t[:, 0:qlen], in_=rsum[:, 0:qlen])
                        rb_t = sb.tile([64, QCS], f32, name=f"rb{qc}_{q0}_{h}",
                                       tag="rb", bufs=2)
                        nc.gpsimd.partition_broadcast(rb_t[:, 0:qlen],
                                                      rs_t[:, 0:qlen])
                        # the very last head pair's normalize gates the tail
                        # cproj: emit it in two column halves so cproj(3)'s
                        # first token tiles start during the second half
                        nsp = 2 if (qc == QCN - 1 and hp == 3) else 1
                        for sp in range(nsp):
                            lo = q0 + sp * qlen // nsp
                            hi = q0 + (sp + 1) * qlen // nsp
                            nc.vector.tensor_tensor(
                                out=at_tiles[hp][half:half + 64, lo:hi],
                                in0=acc[h][0:64, lo - q0:hi - q0],
                                in1=rb_t[:, lo - q0:hi - q0],
                                op=mybir.AluOpType.mult)
                while fi < len(fillers):
                    fillers[fi]()
                    fi += 1

            # PE warmup: dummy matmuls so the HAM clock gate is released
            # before the first real GEMM phase. 10 cold matmuls ~= 4.3us
            # covers the ~3.4us HAM busy window without delaying qkv(0)
            # (bf16 halved the startup weight/x DMAs).
            # memset on DVE, not gpsimd: the gpsimd q7 path (after the
            # ~4us NEFF preamble) was gating the first warmup matmul
            wrm = sb.tile([P, QCS], bf16, name="wrm", tag="wrm", bufs=1)
            nc.vector.memset(wrm, 0.0)
            for w in range(10):
                wps = pp.tile([P, QCS], f32, name=f"wps{w}", tag="mm1",
                              bufs=2)
                nc.tensor.matmul(wps[:, :], wrm[:, 0:128], wrm[:, :],
                                 start=True, stop=True)

            # qkv(0) standalone, then attention(qc) interleaved with
            # qkv(qc+1) and cproj(qc-1)
            for u in qkv_units(0):
                u()
            # preload the exp table-set (~2.7us ACT_TABLE_LOAD) here, during
            # qkv(0)'s matmuls, instead of at attention(0)'s first real exp.
            # Emitted AFTER qkv(0) so the table load can't delay the wq/wp
            # DMA triggers that share the ACT instruction stream.
            wscr = sb.tile([1, 32], f32, name="wscr", tag="wscr", bufs=1)
            nc.scalar.activation(out=wscr, in_=wrm[0:1, 0:32], func=Exp,
                                 scale=0.125)
            for qc in range(QCN - 1):
                a = list(cproj_units(qc - 1)) if qc > 0 else []
                b = list(qkv_units(qc + 1))
                fillers = []
                while a or b:
                    if a:
                        fillers.append(a.pop(0))
                    if b:
                        fillers.append(b.pop(0))
                emit_attention(qc, fillers,
                               rate=(3.0, 2.6, 2.2)[qc])
            # last chunk: attention(3) is ACT-paced with only cproj(2) as
            # real filler work — it runs dry a third of the way in, PE duty
            # thins, and the HAM clock gate halves the PE clock (which then
            # makes the steps PE-bound at the throttled rate). Pad the
            # filler list with dependency-free dummy matmuls and spread
            # everything evenly (rate 1.0 over ~72 steps). The RS has ~20us
            # FIXED cost (size-independent) so splitting it buys nothing —
            # one RS for the whole chunk.
            def dummy_units(n, pfx):
                for w in range(n):
                    def mmk(w=w):
                        wps = pp.tile([P, QCS], f32, name=f"{pfx}{w}",
                                      tag="mm1", bufs=2)
                        nc.tensor.matmul(wps[:, :], wrm[:, 0:128], wrm[:, :],
                                         start=True, stop=True)
                    yield mmk
            emit_attention(QCN - 1, list(cproj_units(QCN - 2))
                           + list(dummy_units(24, "ka")), rate=1.0)
            for u in cproj_units(QCN - 1):
                u()
            # PE keepalive through the tail collective: dummy matmuls keep
            # the HAM clock at full speed while the RS and out DMAs drain
            for u in dummy_units(28, "tws"):
                u()
            # final copies of reduced shards (bf16, upcast on host — the RS
            # output is already bf16 so a f32 cast-DMA adds no precision) on
            # the SYNC queue: its x-prefetch DMAs are long done, so an RS-
            # completion wait here blocks nothing. (On the gpsimd queue the
            # chunk-2 wait stalled attention(3)'s partition_broadcasts for
            # ~19us.) Still pinned after the RS triggers so the scheduler
            # can't hoist a wait ahead of its collective's trigger.
            from concourse.tile import add_dep_helper
            for q in range(QCN):
                di = nc.sync.dma_start(
                    out=out_e.ap()[q * 256:(q + 1) * 256, :],
                    in_=rsos[q][:, :])
                anchor = rs_insts[QCN - 2] if q < QCN - 1 else rs_insts[-1]
                add_dep_helper(di.ins, anchor.ins, sync=False,
                               reason="order out DMAs after RS triggers")
    nc.compile()
    return nc


def _get_nc():
    if "nc" not in _CACHE:
        _CACHE["nc"] = _build()
    return _CACHE["nc"]


def _wlayout(w):
    """[D, ACH] -> [P, 4*FKT*P]: per 128-col tile ct, "(k p) c -> p (k c)"."""
    cts = []
    for ct in range(4):
        blk = w[:, ct * P:(ct + 1) * P].reshape(FKT, P, P)
        cts.append(blk.transpose(1, 0, 2).reshape(P, FKT * P))
    return np.ascontiguousarray(np.concatenate(cts, axis=1))

def _in_maps(x, c_attn_w, c_proj_w):
    import ml_dtypes
    ch = np.arange(P)[:, None]
    co = np.arange(QCS)[None, :]
    ms = []
    for j in range(4):
        m = (co >= j * P + ch).astype(np.float32)
        ms.append(np.concatenate([m, m], axis=1))
    cmask = np.stack(ms, axis=1).astype(ml_dtypes.bfloat16)
    maps = []
    for c in range(NCORES):
        b, g = c // 2, c % 2
        h0 = g * HPC
        cols = slice(h0 * HD, h0 * HD + ACH)
        maps.append({
            "xt": np.ascontiguousarray(x[b].T).astype(ml_dtypes.bfloat16),
            "wq": _wlayout(c_attn_w[:, :D][:, cols]).astype(ml_dtypes.bfloat16),
            "wk": _wlayout(c_attn_w[:, D:2 * D][:, cols]).astype(
                ml_dtypes.bfloat16),
            "wv": np.ascontiguousarray(c_attn_w[:, 2 * D:][:, cols]).astype(
                ml_dtypes.bfloat16),
            "wp": np.ascontiguousarray(
                c_proj_w[h0 * HD:h0 * HD + ACH, :]).astype(ml_dtypes.bfloat16),
            "cmask": cmask,
        })
    return maps


def _run(inputs, trace=False):
    from concourse.bass_utils import run_bass_kernel_spmd
    x = np.asarray(inputs["x"], np.float32)
    c_attn_w = np.asarray(inputs["c_attn_w"], np.float32)
    c_attn_b = np.asarray(inputs["c_attn_b"], np.float32)
    c_proj_w = np.asarray(inputs["c_proj_w"], np.float32)
    c_proj_b = np.asarray(inputs["c_proj_b"], np.float32)
    assert not np.any(c_attn_b), "nonzero c_attn_b not supported"

    nc = _get_nc()
    res = run_bass_kernel_spmd(nc, _in_maps(x, c_attn_w, c_proj_w),
                               core_ids=list(range(NCORES)), trace=trace)
    out = np.empty((B, S, D), np.float32)
    for c in range(NCORES):
        b, g = c // 2, c % 2
        o = np.asarray(res.results[c]["outp"]).astype(np.float32)
        for qc in range(QCN):
            tok = qc * QCS + g * 256
            out[b, tok:tok + 256, :] = o[qc * 256:(qc + 1) * 256]
    if np.any(c_proj_b):
        out += c_proj_b
    return out, res


def kernel(**inputs):
    out, _ = _run(inputs, trace=False)
    return out

